# revision 71
# baseline (speedup 1.0000x reference)
"""Trainium2 Bass kernel for a custom attention block (qkv-proj + LN(q,k) +
RoPE + causal attention + out-proj), distributed over 8 NeuronCores.

Sharding: 2 cores per batch (B=4), split by HEADS (Megatron-style). Core
role r=c%2 owns heads 8r..8r+7: it projects q/k/v only for those 1024
features, runs full causal attention over all 2048 tokens for its 8
heads, and computes a PARTIAL out-projection with its 1024-row slice of
w_out. The two partial [D, S] outputs per batch are summed on the host
during unsharding. LayerNorm over the full 2048 q/k features needs the
peer's sum-of-squares: one tiny 8KB pair-wise AllReduce per projection
(replica groups [[0,1],[2,3],[4,5],[6,7]]), hidden under the k/v
projections. Weight centering on the host makes projected q/k
zero-mean, so LN needs only the second moment.

All matmuls run in bf16 (fp32 PSUM accumulation). x, q and k are
SBUF-resident; v is computed feature-major (w-stationary, 512-wide
moving tiles), round-trips through DRAM, and is transposed to
token-major on the fly by DMA-transpose loads during attention. Causal
structure is exact at 128-kv-chunk granularity: q tile t (512 tokens)
only touches kv chunks 0..4t+3, with elementwise masks on the 4
diagonal chunks (the mask pattern is identical for every tile and
head). The attention-phase ot/vsl buffers are carved out of low chunks
of the (by then dead) resident-x region to fit SBUF.

Engine split: PE does projections/scores/PV/out-proj plus tiny
reduce/broadcast matmuls; Scalar does exp, squares and PSUM->SBUF
copies; DVE does LN/rope muls, masks and the late half of the softmax
denominator accumulation; GpSimd does the early half plus LN
scale-bias, rope adds, sumsq chains and the collective.
"""

import math

import numpy as np

import concourse.bass as bass
import concourse.mybir as mybir
import concourse.tile as tile
from concourse import bacc
from concourse.bass import ds

F32 = mybir.dt.float32
F32R = mybir.dt.float32r
BF16 = mybir.dt.bfloat16
AF = mybir.ActivationFunctionType
OP = mybir.AluOpType

P = 128
HD = 128
D = 2048
S = 2048
NH = D // HD          # 16 global heads
NHL = 8               # heads per core
DC = D // P           # 16 contraction chunks
QT = 512              # q/attention tile width (moving dim)
NT = S // QT          # 4 q tiles (full sequence per core)
NSLAB = S // QT       # 4 projection token slabs
EXP_BIAS = 8.0
EPS = 1e-5
LOOKAHEAD = 3         # attention score-slot software pipeline depth
REPLICA_GROUPS = [[0, 1], [2, 3], [4, 5], [6, 7]]


def _r(ap):
    """fp32 -> fp32r view for matmul operands."""
    return ap.bitcast(F32R)


def _v3(ap):
    """[P, n*128] AP -> [P, n, 128] view (avoids 1-free-dim DMA splits)."""
    return ap.rearrange("p (a x) -> p a x", x=P)


def build_program():
    nc = bacc.Bacc("TRN2", target_bir_lowering=False, debug=False,
                   num_devices=8)

    # ---- I/O ----
    xT_i = nc.dram_tensor("xT", [D, S], BF16, kind="ExternalInput").ap()
    wqk_i = nc.dram_tensor("wqk", [2 * NHL, P, DC, P], BF16,
                           kind="ExternalInput").ap()
    wv_i = nc.dram_tensor("wv", [NHL, P, DC, P], BF16,
                          kind="ExternalInput").ap()
    wo_i = nc.dram_tensor("wo", [NH, P, NHL, P], BF16,
                          kind="ExternalInput").ap()
    cos_i = nc.dram_tensor("cos", [HD, S], BF16, kind="ExternalInput").ap()
    sin_i = nc.dram_tensor("sin", [HD, S], BF16, kind="ExternalInput").ap()
    masks_i = nc.dram_tensor("masks", [P, 4, QT], BF16,
                             kind="ExternalInput").ap()
    onesc_i = nc.dram_tensor("onesc", [P, 1], F32, kind="ExternalInput").ap()
    onesr_i = nc.dram_tensor("onesr", [1, P], F32, kind="ExternalInput").ap()
    rotm_i = nc.dram_tensor("rotm", [P, P], BF16, kind="ExternalInput").ap()
    out_t = nc.dram_tensor("out", [D, S], F32, kind="ExternalOutput").ap()

    with tile.TileContext(nc) as tc:
        import contextlib

        ctx = contextlib.ExitStack()
        with ctx:
            sb = ctx.enter_context(tc.tile_pool(name="sb", bufs=1))
            psum = ctx.enter_context(tc.tile_pool(name="ps", bufs=1, space="PSUM"))
            dram = ctx.enter_context(tc.tile_pool(name="dram", bufs=1, space="DRAM"))

            # ---- DRAM scratch: v (this core's 8 heads), feature-major ----
            vT = dram.tile([NHL * P, S], BF16, tag="vT", name="vT")

            # ---- collective bounce buffers (internal DRAM) ----
            cc_in = {qk: dram.tile([1, NSLAB, QT], F32, tag=f"cc_in_{qk}",
                                   name=f"cc_in_{qk}") for qk in ("q", "k")}
            cc_out = {qk: dram.tile([1, NSLAB, QT], F32, tag=f"cc_out_{qk}",
                                    name=f"cc_out_{qk}") for qk in ("q", "k")}

            # ---- resident x: [128, DC, S] bf16 (64KB/partition) ----
            # x is the startup critical path: spread the chunk loads over
            # both hardware DGE queues (sync + scalar), with the first two
            # weight tiles prefetched on scalar so the PE starts early.
            xsb = sb.tile([P, DC, S], BF16, tag="xsb", bufs=1, name="xsb")
            w_pre = []
            for ec in range(2):
                w = sb.tile([P, DC, P], BF16, tag="w", bufs=5, name="w")
                nc.scalar.dma_start(w, wqk_i[ec])
                w_pre.append(w)
            for d in range(DC):
                eng = nc.sync if d % 2 == 0 else nc.scalar
                eng.dma_start(xsb[:, d], xT_i[ds(d * P, P), :])

            # ---- constants / small inputs (off the x critical path) ----
            ones_col = sb.tile([P, 1], F32, tag="ones_col", name="ones_col")
            nc.sync.dma_start(_r(ones_col), _r(onesc_i))
            ones_row = sb.tile([1, P], F32, tag="ones_row", name="ones_row")
            nc.sync.dma_start(_r(ones_row), _r(onesr_i))
            eps1 = sb.tile([1, 1], F32, tag="eps1", name="eps1")
            nc.vector.memset(eps1, EPS)
            zero1 = sb.tile([1, 1], F32, tag="zero1", name="zero1")
            nc.vector.memset(zero1, 0.0)
            nbias = sb.tile([P, 1], F32, tag="nbias", name="nbias")
            nc.vector.memset(nbias, -EXP_BIAS)
            ones_bf = sb.tile([P, 1], BF16, tag="ones_bf", name="ones_bf")
            nc.vector.memset(ones_bf, 1.0)
            rotm = sb.tile([P, P], BF16, tag="rotm", name="rotm")
            nc.sync.dma_start(rotm, rotm_i)
            cos_t = sb.tile([HD, S], BF16, tag="cos_t", name="cos_t")
            nc.scalar.dma_start(_v3(cos_t), _v3(cos_i))
            sin_t = sb.tile([HD, S], BF16, tag="sin_t", name="sin_t")
            nc.scalar.dma_start(_v3(sin_t), _v3(sin_i))

            # attention-phase buffers carved from xsb's low chunks (those
            # chunks' x data is last READ early inside the final v
            # projection group, so the WAR dependency resolves early).
            vsl_views = [
                xsb[:, c].rearrange("p (c x) -> p c x", x=HD)
                for c in (0, 1)
            ]  # 2 x [P, 16, HD] bf16
            ot_views = [
                xsb[:, ds(c, 2)].rearrange("p a (h q) -> p (a h) q", q=QT)
                for c in (2, 4)
            ]  # 2 x [P, 8, QT] bf16

            # ---- resident post-LN+rope q/k ----
            q_res = sb.tile([P, NHL, S], BF16, tag="q_res", bufs=1,
                            name="q_res")
            k_res = sb.tile([P, NHL, S], BF16, tag="k_res", bufs=1,
                            name="k_res")

            def proj_group(ec, dst, sqsums):
                """One 128-feature projection chunk for all 4 token slabs.
                w-stationary: each [128,128] weight tile streams 4 moving
                slabs. dst: [P, S] AP (q/k head chunk) or None for v
                (which goes feature-major to DRAM). sqsums accumulates
                squares when given."""
                if ec < 2:
                    w = w_pre[ec]
                else:
                    w = sb.tile([P, DC, P], BF16, tag="w", bufs=5, name="w")
                    nc.sync.dma_start(w, wqk_i[ec] if ec < 2 * NHL
                                      else wv_i[ec - 2 * NHL])
                pss = [psum.tile([P, QT], F32, tag="mm", bufs=5, name="ps")
                       for _ in range(NSLAB)]
                for d in range(DC):
                    for i in range(NSLAB):
                        nc.tensor.matmul(
                            pss[i],
                            lhsT=w[:, d],
                            rhs=xsb[:, d, ds(i * QT, QT)],
                            start=(d == 0),
                            stop=(d == DC - 1),
                        )
                for i in range(NSLAB):
                    if dst is None:
                        f = ec - 2 * NHL
                        vtsb = sb.tile([P, QT], BF16, tag="vsb", bufs=2,
                                       name="vtsb")
                        nc.scalar.copy(vtsb, pss[i])
                        nc.gpsimd.dma_start(
                            _v3(vT[ds(f * P, P), ds(i * QT, QT)]), _v3(vtsb)
                        )
                    else:
                        nc.scalar.copy(dst[:, ds(i * QT, QT)], pss[i])
                    if sqsums is not None:
                        sq = sb.tile([P, QT], BF16, tag="sq", bufs=2,
                                     name="sq")
                        nc.scalar.square(sq, pss[i])
                        if sqsums[i] is None:
                            sqsums[i] = sb.tile([P, QT], F32, tag="acc",
                                                bufs=8, name="sqsum")
                            nc.vector.tensor_copy(_r(sqsums[i]), sq)
                        else:
                            nc.gpsimd.tensor_tensor(_r(sqsums[i]), sqsums[i],
                                                    sq, op=OP.add)

            def emit_stats(qk, sqsums):
                """Partition-reduce the per-slab sumsq and launch the
                pair AllReduce (each core of a batch owns half the
                features)."""
                for i in range(NSLAB):
                    pstat = psum.tile([1, QT], F32, tag="stat", bufs=3,
                                      name="pstat")
                    nc.tensor.matmul(pstat, lhsT=_r(ones_col),
                                     rhs=_r(sqsums[i]))
                    st = sb.tile([1, QT], F32, tag="st", bufs=1, name="st")
                    nc.scalar.copy(st, pstat)
                    nc.gpsimd.dma_start(cc_in[qk][:, i], st)
                nc.gpsimd.collective_compute(
                    "AllReduce",
                    OP.add,
                    replica_groups=REPLICA_GROUPS,
                    ins=[cc_in[qk][:].opt()],
                    outs=[cc_out[qk][:].opt()],
                )

            def tail_head(qk, i):
                """LN-statistics prologue for one slab: read back the
                AllReduced sumsq, rsig = exp(-0.5*ln(sumsq/D+eps)), and
                broadcast it across partitions. Emitted BEFORE a
                v-projection group so the tail_body's DVE/GpSimd work can
                run under that group's matmul stream."""
                str_ = sb.tile([1, QT], F32, tag="str", bufs=2, name="str")
                nc.scalar.dma_start(str_, cc_out[qk][:, i])
                lnv = sb.tile([1, QT], F32, tag="stats_sb", bufs=2,
                              name="lnv")
                # q's projection rows carry the folded 1/sqrt(HD) score
                # scale, so its sumsq is scaled by 1/HD
                nc.scalar.activation(lnv, str_, AF.Ln,
                                     scale=(HD / D if qk == "q" else 1.0 / D),
                                     bias=eps1)
                rsig = sb.tile([1, QT], F32, tag="stats_sb", bufs=2,
                               name="rsig")
                nc.scalar.activation(_r(rsig), lnv, AF.Exp,
                                     bias=zero1, scale=-0.5)
                ps_rep = psum.tile([P, QT], F32, tag="mm", bufs=5,
                                   name="ps_rep")
                nc.tensor.matmul(ps_rep, lhsT=_r(ones_row), rhs=_r(rsig))
                return (qk, i, ps_rep)

            def tail_body(pend):
                """LN + rope for one token slab x 8 head chunks. The LN
                gamma (and the 1/sqrt(hd) score scale for q) are folded
                into the projection weights on the host; beta is zero by
                the problem spec, so LN is a single per-token multiply."""
                qk, i, ps_rep = pend
                res = q_res if qk == "q" else k_res
                csl = ds(i * QT, QT)
                for ec in range(NHL):
                    ch = res[:, ec, csl]
                    nc.vector.tensor_tensor(ch, ch, ps_rep, op=OP.mult)
                # pass 2: rope; rotation matmuls stream back-to-back
                for ec in range(NHL):
                    ch = res[:, ec, csl]
                    ps_rot = psum.tile([P, QT], F32, tag="mm",
                                       bufs=5, name="ps_rot")
                    nc.tensor.matmul(ps_rot, lhsT=rotm, rhs=ch)
                    tmp = sb.tile([P, QT], BF16, tag="rtmp", bufs=2,
                                  name="rtmp")
                    nc.vector.tensor_tensor(tmp, ps_rot,
                                            sin_t[:, csl], op=OP.mult)
                    nc.vector.tensor_tensor(ch, ch, cos_t[:, csl],
                                            op=OP.mult)
                    nc.gpsimd.tensor_tensor(ch, ch, tmp, op=OP.add)

            # ---- Projections ----
            sq_q = [None] * NSLAB
            sq_k = [None] * NSLAB
            for ec in range(NHL):                       # q: heads 0..7
                proj_group(ec, q_res[:, ec], sq_q)
            proj_group(NHL, k_res[:, 0], sq_k)          # k head 0
            emit_stats("q", sq_q)
            for ec in range(1, NHL):                    # k: heads 1..7
                proj_group(NHL + ec, k_res[:, ec], sq_k)
            proj_group(2 * NHL, None, None)             # v head 0
            emit_stats("k", sq_k)
            # LN+rope tails, software-pipelined one slab per v group:
            # each slab's rsig broadcast (PE) is emitted BEFORE the next v
            # group, its DVE/GpSimd body after it.
            # k slab i is first needed by attention tile i, q slab i by
            # tile i only — so the late tails are the late-needed ones,
            # and the final q3 body's DVE work drains under early
            # attention tiles (which never touch q slab 3).
            # attention runs tiles in order 1,2,3,0 — so q slab 1 is
            # needed first and q slab 0 dead last; k slab i is first
            # needed by tile i
            tails = [("q", 1), ("k", 0), ("k", 1), ("k", 2),
                     ("k", 3), ("q", 2), ("q", 3), ("q", 0)]
            pend = tail_head(*tails[0])
            for f in range(1, NHL):                     # v heads 1..7
                proj_group(2 * NHL + f, None, None)
                tail_body(pend)
                pend = tail_head(*tails[f])
            tail_body(pend)                             # q slab 3
            # the causal masks overlay the (now dead) cos buffer; loaded
            # on the scalar DMA queue (its last entry) so the wait for
            # cos to die doesn't head-of-line-block anything
            masks = sb.tile([P, 4, QT], BF16, tag="cos_t", name="masks")
            nc.scalar.dma_start(masks, masks_i)

            # ---- Attention + out-projection per q tile ----
            def outproj_chunk(ot_src, qoff, e):
                """One out-projection feature chunk; pure PE work used to
                fill exp-latency gaps in the attention stream."""
                wot = sb.tile([P, NHL, P], BF16, tag="wot", bufs=2,
                              name="wot")
                nc.sync.dma_start(wot, wo_i[e])
                psf = psum.tile([P, QT], F32, tag="stat", bufs=3,
                                name="psf")
                for h in range(NHL):
                    nc.tensor.matmul(
                        psf,
                        lhsT=wot[:, h],
                        rhs=ot_src[:, h],
                        start=(h == 0),
                        stop=(h == NHL - 1),
                    )
                fsb = sb.tile([P, QT], F32, tag="fsb", bufs=2,
                              name="fsb")
                nc.vector.tensor_copy(fsb, psf)
                nc.scalar.dma_start(
                    _v3(out_t[ds(e * P, P), ds(qoff, QT)]), _v3(fsb)
                )

            # tile order 1,2,3,0: a bare first tile (no out-projection
            # filler yet) is cheapest at 8 slots, and tile 0 — which has
            # the least PE work per head — gets tile 3's out-projection
            # to stay dense
            prev_ot = None
            prev_qoff = None
            for t in (1, 2, 3, 0):
                qsl_off = t * QT
                n_slots = 4 * (t + 1)
                ot_res = ot_views[t % 2]

                def norm_a(pending):
                    """Reciprocal of the PE-accumulated softmax
                    denominator. The PE consumer (psr) is in norm_b,
                    emitted a full head later, so the PE never waits on
                    the DVE reciprocal."""
                    psout_p, psden, h_p = pending
                    rec0 = sb.tile([1, QT], F32, tag="stats_sb", bufs=2,
                                   name="rec0")
                    with nc.allow_low_precision(
                        reason="denominator reciprocal, 18 bits is plenty"
                    ):
                        nc.vector.reciprocal_approx_fast(rec0, psden)
                    rec = sb.tile([1, QT], F32, tag="rec", bufs=2,
                                  name="rec")
                    nc.vector.tensor_copy(_r(rec), rec0)
                    return (psout_p, rec, h_p)

                def norm_b(normed):
                    psout_p, rec, h_p = normed
                    psr = psum.tile([P, QT], F32, tag="stat", bufs=3,
                                    name="psr")
                    nc.tensor.matmul(psr, lhsT=_r(ones_row), rhs=_r(rec))
                    nc.vector.tensor_copy(ot_res[:, h_p], psout_p)
                    nc.vector.tensor_tensor(ot_res[:, h_p], ot_res[:, h_p],
                                            psr, op=OP.mult)

                ready = []
                for h in range(NHL):
                    vsl = vsl_views[(t * NHL + h) % 2]
                    nc.sync.dma_start_transpose(
                        vsl[:, ds(0, n_slots)],
                        vT[ds(h * HD, HD), ds(0, n_slots * P)],
                    )
                    ksl = k_res[:, h].rearrange("p (c x) -> p c x", x=P)
                    # head h-2's psr broadcast FIRST: frees its psout so
                    # at most 2 psout + 3 score tiles are live in the
                    # 5-deep PSUM ring, and the h-2 reciprocal has had a
                    # full head of DVE slack
                    if len(ready) >= 2:
                        norm_b(ready.pop(0))
                    psout = psum.tile([P, QT], F32, tag="mm", bufs=5,
                                      name="psout")
                    psden = psum.tile([1, QT], F32, tag="stat", bufs=3,
                                      name="psden")
                    qsl = q_res[:, h, ds(qsl_off, QT)]

                    ets = {}

                    def emit_score(s):
                        # causal trim: diagonal kv chunk i only serves q
                        # columns >= 128*i; its first 128 columns get the
                        # (shared) triangular mask
                        diag = s - 4 * t
                        qo = 128 * diag if diag > 0 else 0
                        wq_ = QT - qo
                        pss = psum.tile([P, QT], F32, tag="mm", bufs=5,
                                        name="pss")
                        nc.tensor.matmul(pss[:, ds(0, wq_)], lhsT=ksl[:, s],
                                         rhs=qsl[:, ds(qo, wq_)])
                        et = sb.tile([P, QT], BF16, tag="exp", bufs=4,
                                     name="et")
                        nc.scalar.activation(et[:, ds(0, wq_)],
                                             pss[:, ds(0, wq_)], AF.Exp,
                                             bias=nbias[:, ds(0, 1)])
                        if diag >= 0:
                            nc.vector.tensor_tensor(et[:, ds(0, P)],
                                                    et[:, ds(0, P)],
                                                    masks[:, 0, ds(0, P)],
                                                    op=OP.mult)
                        ets[s] = (et, qo, wq_)

                    # scores first so Scalar starts on the exps, THEN the
                    # previous tile's out-projection (pure PE work) fills
                    # the exp latency before the PV stream needs et(0)
                    for s in range(min(LOOKAHEAD, n_slots)):
                        emit_score(s)
                    if prev_ot is not None:
                        outproj_chunk(prev_ot, prev_qoff, 2 * h)
                        outproj_chunk(prev_ot, prev_qoff, 2 * h + 1)
                    for s in range(n_slots):
                        if s + LOOKAHEAD < n_slots:
                            emit_score(s + LOOKAHEAD)
                        et, qo, wq_ = ets.pop(s)
                        nc.tensor.matmul(
                            psout[:, ds(qo, wq_)],
                            lhsT=vsl[:, s],
                            rhs=et[:, ds(0, wq_)],
                            start=(s == 0),
                            stop=(s == n_slots - 1),
                        )
                        # softmax denominator: PE PSUM-accumulated ones
                        # reduction (the vector engines can't keep up
                        # with chained adds; the PE absorbs it for 512
                        # rows/slot with no cross-engine serialization)
                        nc.tensor.matmul(
                            psden[:, ds(qo, wq_)],
                            lhsT=ones_bf,
                            rhs=et[:, ds(0, wq_)],
                            start=(s == 0),
                            stop=(s == n_slots - 1),
                        )
                    # this head's reciprocal: DVE has all of head h+1 to
                    # finish it before norm_b(h) runs at the top of h+2
                    ready.append(norm_a((psout, psden, h)))
                for r in ready:
                    norm_b(r)
                prev_ot = ot_res
                prev_qoff = qsl_off

            # ---- out-projection for the last q tile ----
            for e in range(NH):
                outproj_chunk(prev_ot, prev_qoff, e)

    nc.compile()
    return nc


# --------------------------------------------------------------------------
# Host-side prep and driver
# --------------------------------------------------------------------------


def make_host_data(x, w_in, w_out, q_gamma, q_beta, k_gamma, k_beta):
    """Build per-core in_maps (list of dicts). Core c: batch c//2, heads
    (c%2)*8..(c%2)*8+7."""
    import ml_dtypes
    bf16 = ml_dtypes.bfloat16

    B = x.shape[0]
    n_cores = 2 * B
    HF = NHL * P  # 1024 local head-features

    # LN gamma/beta are ones/zeros by the problem spec; the kernel folds
    # the q-side 1/sqrt(HD) score scale into the q projection rows and
    # drops the affine entirely.
    assert np.allclose(np.asarray(q_gamma, np.float32), 1.0)
    assert np.allclose(np.asarray(k_gamma, np.float32), 1.0)
    assert np.allclose(np.asarray(q_beta, np.float32), 0.0)
    assert np.allclose(np.asarray(k_beta, np.float32), 0.0)

    w32 = np.asarray(w_in, np.float32)
    wq = w32[0:D]
    wk = w32[D:2 * D]
    wv = w32[2 * D:3 * D]
    # centering the output features makes projected q/k exactly zero-mean
    wq_c = (wq - wq.mean(axis=0, keepdims=True)) / math.sqrt(HD)
    wk_c = wk - wk.mean(axis=0, keepdims=True)
    woT = np.asarray(w_out, np.float32).T  # [D(hfeat), D(out)]

    inv = 1.0 / (10000.0 ** (np.arange(0, HD, 2, dtype=np.float64) / HD))
    tpos = np.arange(S, dtype=np.float64)
    fr = np.outer(tpos, inv)
    emb = np.concatenate([fr, fr], axis=-1)  # [S, HD]
    cosT = np.ascontiguousarray(np.cos(emb).T).astype(bf16)  # [HD, S]
    sinT = np.ascontiguousarray(np.sin(emb).T).astype(bf16)

    h2 = HD // 2
    rotmT = np.zeros((P, P), np.float32)
    for p in range(h2):
        rotmT[p + h2, p] = -1.0
    for p in range(h2, HD):
        rotmT[p - h2, p] = 1.0
    rotm = rotmT.astype(bf16)

    # diagonal causal masks, identical for every 512-token tile:
    # mask[p, i, j] = (128*i + p) <= j
    pp = np.arange(P)[:, None, None]
    ii = np.arange(4)[None, :, None]
    jj = np.arange(QT)[None, None, :]
    masks = ((128 * ii + pp) <= jj).astype(np.float32).astype(bf16)

    scale = 1.0 / math.sqrt(HD)
    onesc = np.ones((P, 1), np.float32)
    onesr = np.ones((1, P), np.float32)

    role_data = []
    for r in range(2):
        fsl = slice(r * HF, (r + 1) * HF)
        wqkT = np.concatenate([wq_c[fsl].T, wk_c[fsl].T], axis=1)  # [D, 2HF]
        wqk_t = np.ascontiguousarray(
            wqkT.reshape(DC, P, 2 * NHL, P).transpose(2, 1, 0, 3)
        ).astype(bf16)
        wv_t = np.ascontiguousarray(
            wv[fsl].T.reshape(DC, P, NHL, P).transpose(2, 1, 0, 3)
        ).astype(bf16)
        wo_t = np.ascontiguousarray(
            woT[fsl].reshape(NHL, P, NH, P).transpose(2, 1, 0, 3)
        ).astype(bf16)
        role_data.append(dict(wqk=wqk_t, wv=wv_t, wo=wo_t))

    in_maps = []
    xT_b = {}
    for c in range(n_cores):
        b, r = c // 2, c % 2
        if b not in xT_b:
            xT_b[b] = np.ascontiguousarray(
                np.asarray(x[b], np.float32).T
            ).astype(bf16)
        in_maps.append(dict(
            xT=xT_b[b], cos=cosT, sin=sinT, masks=masks,
            onesc=onesc, onesr=onesr, rotm=rotm,
            **role_data[r],
        ))
    return in_maps


_PROGRAM_CACHE = {}


def _get_program():
    if "full" not in _PROGRAM_CACHE:
        _PROGRAM_CACHE["full"] = build_program()
    return _PROGRAM_CACHE["full"]


def run_full(x, w_in, w_out, q_gamma, q_beta, k_gamma, k_beta,
             trace=False):
    from concourse.bass_utils import run_bass_kernel_spmd

    B = x.shape[0]
    n_cores = 2 * B
    in_maps = make_host_data(
        x, w_in, w_out, q_gamma, q_beta, k_gamma, k_beta,
    )
    nc = _get_program()
    res = run_bass_kernel_spmd(
        nc, in_maps, core_ids=list(range(n_cores)), trace=trace,
    )
    out = np.empty((B, S, D), np.float32)
    for b in range(B):
        o = res.results[2 * b]["out"] + res.results[2 * b + 1]["out"]
        out[b] = o.T
    return out, res


def kernel(x, w_in, w_out, q_gamma, q_beta, k_gamma, k_beta, n_heads=16,
           **_ignored):
    x = np.asarray(x, np.float32)
    assert int(np.asarray(n_heads)) * HD == x.shape[-1]
    out, _ = run_full(
        np.asarray(x, np.float32),
        np.asarray(w_in, np.float32),
        np.asarray(w_out, np.float32),
        np.asarray(q_gamma, np.float32),
        np.asarray(q_beta, np.float32),
        np.asarray(k_gamma, np.float32),
        np.asarray(k_beta, np.float32),
    )
    return out


# revision 77
# speedup vs baseline: 1.0266x; 1.0266x over previous
"""Trainium2 Bass kernel for a custom attention block (qkv-proj + LN(q,k) +
RoPE + causal attention + out-proj), distributed over 8 NeuronCores.

Sharding: 2 cores per batch (B=4), split by HEADS (Megatron-style). Core
role r=c%2 owns heads 8r..8r+7: it projects q/k/v only for those 1024
features, runs full causal attention over all 2048 tokens for its 8
heads, and computes a PARTIAL out-projection with its 1024-row slice of
w_out. The two partial [D, S] outputs per batch are summed on the host
during unsharding. LayerNorm over the full 2048 q/k features needs the
peer's sum-of-squares: one tiny 8KB pair-wise AllReduce per projection
(replica groups [[0,1],[2,3],[4,5],[6,7]]), hidden under the k/v
projections. Weight centering on the host makes projected q/k
zero-mean, so LN needs only the second moment.

All matmuls run in bf16 (fp32 PSUM accumulation). x, q and k are
SBUF-resident; v is computed feature-major (w-stationary, 512-wide
moving tiles), round-trips through DRAM, and is transposed to
token-major on the fly by DMA-transpose loads during attention. Causal
structure is exact at 128-kv-chunk granularity: q tile t (512 tokens)
only touches kv chunks 0..4t+3, with elementwise masks on the 4
diagonal chunks (the mask pattern is identical for every tile and
head). The attention-phase ot/vsl buffers are carved out of low chunks
of the (by then dead) resident-x region to fit SBUF.

Engine split: PE does projections/scores/PV/out-proj plus tiny
reduce/broadcast matmuls; Scalar does exp, squares and PSUM->SBUF
copies; DVE does LN/rope muls, masks and the late half of the softmax
denominator accumulation; GpSimd does the early half plus LN
scale-bias, rope adds, sumsq chains and the collective.
"""

import math

import numpy as np

import concourse.bass as bass
import concourse.mybir as mybir
import concourse.tile as tile
from concourse import bacc
from concourse.bass import ds

F32 = mybir.dt.float32
F32R = mybir.dt.float32r
BF16 = mybir.dt.bfloat16
AF = mybir.ActivationFunctionType
OP = mybir.AluOpType

P = 128
HD = 128
D = 2048
S = 2048
NH = D // HD          # 16 global heads
NHL = 8               # heads per core
DC = D // P           # 16 contraction chunks
QT = 512              # q/attention tile width (moving dim)
NT = S // QT          # 4 q tiles (full sequence per core)
NSLAB = S // QT       # 4 projection token slabs
EXP_BIAS = 8.0
EPS = 1e-5
LOOKAHEAD = 3         # attention score-slot software pipeline depth
REPLICA_GROUPS = [[0, 1], [2, 3], [4, 5], [6, 7]]


def _r(ap):
    """fp32 -> fp32r view for matmul operands."""
    return ap.bitcast(F32R)


def _v3(ap):
    """[P, n*128] AP -> [P, n, 128] view (avoids 1-free-dim DMA splits)."""
    return ap.rearrange("p (a x) -> p a x", x=P)


def build_program():
    nc = bacc.Bacc("TRN2", target_bir_lowering=False, debug=False,
                   num_devices=8)

    # ---- I/O ----
    xT_i = nc.dram_tensor("xT", [D, S], BF16, kind="ExternalInput").ap()
    wqk_i = nc.dram_tensor("wqk", [2 * NHL, P, DC, P], BF16,
                           kind="ExternalInput").ap()
    wv_i = nc.dram_tensor("wv", [NHL, P, DC, P], BF16,
                          kind="ExternalInput").ap()
    wo_i = nc.dram_tensor("wo", [NH, P, NHL, P], BF16,
                          kind="ExternalInput").ap()
    cos_i = nc.dram_tensor("cos", [HD, S], BF16, kind="ExternalInput").ap()
    sin_i = nc.dram_tensor("sin", [HD, S], BF16, kind="ExternalInput").ap()
    masks_i = nc.dram_tensor("masks", [P, 4, QT], BF16,
                             kind="ExternalInput").ap()
    onesc_i = nc.dram_tensor("onesc", [P, 1], F32, kind="ExternalInput").ap()
    onesr_i = nc.dram_tensor("onesr", [1, P], F32, kind="ExternalInput").ap()
    rotm_i = nc.dram_tensor("rotm", [P, P], BF16, kind="ExternalInput").ap()
    out_t = nc.dram_tensor("out", [D, S], F32, kind="ExternalOutput").ap()

    with tile.TileContext(nc) as tc:
        import contextlib

        ctx = contextlib.ExitStack()
        with ctx:
            sb = ctx.enter_context(tc.tile_pool(name="sb", bufs=1))
            psum = ctx.enter_context(tc.tile_pool(name="ps", bufs=1, space="PSUM"))
            dram = ctx.enter_context(tc.tile_pool(name="dram", bufs=1, space="DRAM"))

            # ---- DRAM scratch: v (this core's 8 heads), feature-major ----
            vT = dram.tile([NHL * P, S], BF16, tag="vT", name="vT")

            # ---- collective bounce buffers (internal DRAM) ----
            cc_in = {qk: dram.tile([1, NSLAB, QT], F32, tag=f"cc_in_{qk}",
                                   name=f"cc_in_{qk}") for qk in ("q", "k")}
            cc_out = {qk: dram.tile([1, NSLAB, QT], F32, tag=f"cc_out_{qk}",
                                    name=f"cc_out_{qk}") for qk in ("q", "k")}

            # ---- resident x: [128, DC, S] bf16 (64KB/partition) ----
            # x is the startup critical path: spread the chunk loads over
            # both hardware DGE queues (sync + scalar), with the first two
            # weight tiles prefetched on scalar so the PE starts early.
            xsb = sb.tile([P, DC, S], BF16, tag="xsb", bufs=1, name="xsb")
            w_pre = []
            for ec in range(2):
                w = sb.tile([P, DC, P], BF16, tag="w", bufs=5, name="w")
                nc.scalar.dma_start(w, wqk_i[ec])
                w_pre.append(w)
            for d in range(DC):
                eng = nc.sync if d % 2 == 0 else nc.scalar
                eng.dma_start(xsb[:, d], xT_i[ds(d * P, P), :])

            # ---- constants / small inputs (off the x critical path) ----
            ones_col = sb.tile([P, 1], F32, tag="ones_col", name="ones_col")
            nc.sync.dma_start(_r(ones_col), _r(onesc_i))
            ones_row = sb.tile([1, P], F32, tag="ones_row", name="ones_row")
            nc.sync.dma_start(_r(ones_row), _r(onesr_i))
            eps1 = sb.tile([1, 1], F32, tag="eps1", name="eps1")
            nc.vector.memset(eps1, EPS)
            zero1 = sb.tile([1, 1], F32, tag="zero1", name="zero1")
            nc.vector.memset(zero1, 0.0)
            nbias = sb.tile([P, 1], F32, tag="nbias", name="nbias")
            nc.vector.memset(nbias, -EXP_BIAS)
            ones_bf = sb.tile([P, 1], BF16, tag="ones_bf", name="ones_bf")
            nc.vector.memset(ones_bf, 1.0)
            rotm = sb.tile([P, P], BF16, tag="rotm", name="rotm")
            nc.sync.dma_start(rotm, rotm_i)
            cos_t = sb.tile([HD, S], BF16, tag="cos_t", name="cos_t")
            nc.scalar.dma_start(_v3(cos_t), _v3(cos_i))
            sin_t = sb.tile([HD, S], BF16, tag="sin_t", name="sin_t")
            nc.scalar.dma_start(_v3(sin_t), _v3(sin_i))

            # attention-phase buffers carved from xsb's low chunks (those
            # chunks' x data is last READ early inside the final v
            # projection group, so the WAR dependency resolves early).
            vsl_views = [
                xsb[:, c].rearrange("p (c x) -> p c x", x=HD)
                for c in (0, 1)
            ]  # 2 x [P, 16, HD] bf16
            ot_views = [
                xsb[:, ds(c, 2)].rearrange("p a (h q) -> p (a h) q", q=QT)
                for c in (2, 4)
            ]  # 2 x [P, 8, QT] bf16

            # ---- resident post-LN+rope q/k ----
            q_res = sb.tile([P, NHL, S], BF16, tag="q_res", bufs=1,
                            name="q_res")
            k_res = sb.tile([P, NHL, S], BF16, tag="k_res", bufs=1,
                            name="k_res")

            def proj_group(ec, dst, sqsums):
                """One 128-feature projection chunk for all 4 token slabs.
                w-stationary: each [128,128] weight tile streams 4 moving
                slabs. dst: [P, S] AP (q/k head chunk) or None for v
                (which goes feature-major to DRAM). sqsums accumulates
                squares when given."""
                if ec < 2:
                    w = w_pre[ec]
                else:
                    w = sb.tile([P, DC, P], BF16, tag="w", bufs=5, name="w")
                    nc.sync.dma_start(w, wqk_i[ec] if ec < 2 * NHL
                                      else wv_i[ec - 2 * NHL])
                pss = [psum.tile([P, QT], F32, tag="mm", bufs=5, name="ps")
                       for _ in range(NSLAB)]
                for d in range(DC):
                    for i in range(NSLAB):
                        nc.tensor.matmul(
                            pss[i],
                            lhsT=w[:, d],
                            rhs=xsb[:, d, ds(i * QT, QT)],
                            start=(d == 0),
                            stop=(d == DC - 1),
                        )
                for i in range(NSLAB):
                    if dst is None:
                        f = ec - 2 * NHL
                        vtsb = sb.tile([P, QT], BF16, tag="vsb", bufs=2,
                                       name="vtsb")
                        nc.scalar.copy(vtsb, pss[i])
                        nc.gpsimd.dma_start(
                            _v3(vT[ds(f * P, P), ds(i * QT, QT)]), _v3(vtsb)
                        )
                    else:
                        nc.scalar.copy(dst[:, ds(i * QT, QT)], pss[i])
                    if sqsums is not None:
                        sq = sb.tile([P, QT], BF16, tag="sq", bufs=2,
                                     name="sq")
                        nc.scalar.square(sq, pss[i])
                        if sqsums[i] is None:
                            sqsums[i] = sb.tile([P, QT], F32, tag="acc",
                                                bufs=8, name="sqsum")
                            nc.vector.tensor_copy(_r(sqsums[i]), sq)
                        else:
                            nc.gpsimd.tensor_tensor(_r(sqsums[i]), sqsums[i],
                                                    sq, op=OP.add)

            def emit_stats(qk, sqsums):
                """Partition-reduce the per-slab sumsq and launch the
                pair AllReduce (each core of a batch owns half the
                features)."""
                for i in range(NSLAB):
                    pstat = psum.tile([1, QT], F32, tag="stat", bufs=3,
                                      name="pstat")
                    nc.tensor.matmul(pstat, lhsT=_r(ones_col),
                                     rhs=_r(sqsums[i]))
                    st = sb.tile([1, QT], F32, tag="st", bufs=1, name="st")
                    nc.scalar.copy(st, pstat)
                    nc.gpsimd.dma_start(cc_in[qk][:, i], st)
                nc.gpsimd.collective_compute(
                    "AllReduce",
                    OP.add,
                    replica_groups=REPLICA_GROUPS,
                    ins=[cc_in[qk][:].opt()],
                    outs=[cc_out[qk][:].opt()],
                )

            def tail_head(qk, i):
                """LN-statistics prologue for one slab: read back the
                AllReduced sumsq, rsig = exp(-0.5*ln(sumsq/D+eps)), and
                broadcast it across partitions. Emitted BEFORE a
                v-projection group so the tail_body's DVE/GpSimd work can
                run under that group's matmul stream."""
                str_ = sb.tile([1, QT], F32, tag="str", bufs=2, name="str")
                nc.scalar.dma_start(str_, cc_out[qk][:, i])
                lnv = sb.tile([1, QT], F32, tag="stats_sb", bufs=2,
                              name="lnv")
                # q's projection rows carry the folded 1/sqrt(HD) score
                # scale, so its sumsq is scaled by 1/HD
                nc.scalar.activation(lnv, str_, AF.Ln,
                                     scale=(HD / D if qk == "q" else 1.0 / D),
                                     bias=eps1)
                rsig = sb.tile([1, QT], F32, tag="stats_sb", bufs=2,
                               name="rsig")
                nc.scalar.activation(_r(rsig), lnv, AF.Exp,
                                     bias=zero1, scale=-0.5)
                ps_rep = psum.tile([P, QT], F32, tag="mm", bufs=5,
                                   name="ps_rep")
                nc.tensor.matmul(ps_rep, lhsT=_r(ones_row), rhs=_r(rsig))
                return (qk, i, ps_rep)

            def tail_body(pend):
                """LN + rope for one token slab x 8 head chunks. The LN
                gamma (and the 1/sqrt(hd) score scale for q) are folded
                into the projection weights on the host; beta is zero by
                the problem spec, so LN is a single per-token multiply."""
                qk, i, ps_rep = pend
                res = q_res if qk == "q" else k_res
                csl = ds(i * QT, QT)
                for ec in range(NHL):
                    ch = res[:, ec, csl]
                    nc.vector.tensor_tensor(ch, ch, ps_rep, op=OP.mult)
                # pass 2: rope; rotation matmuls stream back-to-back
                for ec in range(NHL):
                    ch = res[:, ec, csl]
                    ps_rot = psum.tile([P, QT], F32, tag="mm",
                                       bufs=5, name="ps_rot")
                    nc.tensor.matmul(ps_rot, lhsT=rotm, rhs=ch)
                    tmp = sb.tile([P, QT], BF16, tag="rtmp", bufs=2,
                                  name="rtmp")
                    nc.vector.tensor_tensor(tmp, ps_rot,
                                            sin_t[:, csl], op=OP.mult)
                    nc.vector.tensor_tensor(ch, ch, cos_t[:, csl],
                                            op=OP.mult)
                    nc.gpsimd.tensor_tensor(ch, ch, tmp, op=OP.add)

            # ---- Projections ----
            sq_q = [None] * NSLAB
            sq_k = [None] * NSLAB
            for ec in range(NHL):                       # q: heads 0..7
                proj_group(ec, q_res[:, ec], sq_q)
            proj_group(NHL, k_res[:, 0], sq_k)          # k head 0
            emit_stats("q", sq_q)
            for ec in range(1, NHL):                    # k: heads 1..7
                proj_group(NHL + ec, k_res[:, ec], sq_k)
            proj_group(2 * NHL, None, None)             # v head 0
            emit_stats("k", sq_k)
            # LN+rope tails, software-pipelined one slab per v group:
            # each slab's rsig broadcast (PE) is emitted BEFORE the next v
            # group, its DVE/GpSimd body after it.
            # k slab i is first needed by attention tile i, q slab i by
            # tile i only — so the late tails are the late-needed ones,
            # and the final q3 body's DVE work drains under early
            # attention tiles (which never touch q slab 3).
            # attention runs tiles in order 1,2,3,0 — q slab 1 is needed
            # first and q slab 0 dead last; k slab i first at tile i
            tails = [("q", 1), ("k", 0), ("k", 1), ("k", 2),
                     ("k", 3), ("q", 2), ("q", 3), ("q", 0)]
            pend = tail_head(*tails[0])
            for f in range(1, NHL):                     # v heads 1..7
                proj_group(2 * NHL + f, None, None)
                tail_body(pend)
                pend = tail_head(*tails[f])
            tail_body(pend)                             # q slab 3
            # the causal masks overlay the (now dead) cos buffer; loaded
            # on the scalar DMA queue (its last entry) so the wait for
            # cos to die doesn't head-of-line-block anything
            masks = sb.tile([P, 4, QT], BF16, tag="cos_t", name="masks")
            nc.scalar.dma_start(masks, masks_i)

            # ---- Attention + out-projection per q tile ----
            def outproj_chunk(ot_src, qoff, e):
                """One out-projection feature chunk; pure PE work used to
                fill exp-latency gaps in the attention stream."""
                wot = sb.tile([P, NHL, P], BF16, tag="wot", bufs=2,
                              name="wot")
                nc.sync.dma_start(wot, wo_i[e])
                psf = psum.tile([P, QT], F32, tag="stat", bufs=3,
                                name="psf")
                for h in range(NHL):
                    nc.tensor.matmul(
                        psf,
                        lhsT=wot[:, h],
                        rhs=ot_src[:, h],
                        start=(h == 0),
                        stop=(h == NHL - 1),
                    )
                fsb = sb.tile([P, QT], F32, tag="fsb", bufs=2,
                              name="fsb")
                nc.vector.tensor_copy(fsb, psf)
                nc.scalar.dma_start(
                    _v3(out_t[ds(e * P, P), ds(qoff, QT)]), _v3(fsb)
                )

            # tile order 1,2,3,0: a bare first tile (no out-projection
            # filler yet) is cheapest at 8 slots, and tile 0 — the least
            # PE work per head — gets tile 3's out-projection as filler
            prev_ot = None
            prev_qoff = None
            for t in (1, 2, 3, 0):
                qsl_off = t * QT
                n_slots = 4 * (t + 1)
                ot_res = ot_views[t % 2]

                def norm_a(pending):
                    """Reciprocal of the PE-accumulated softmax
                    denominator. The PE consumer (psr) is in norm_b,
                    emitted a full head later, so the PE never waits on
                    the DVE reciprocal."""
                    psout_p, psden, h_p = pending
                    rec0 = sb.tile([1, QT], F32, tag="stats_sb", bufs=2,
                                   name="rec0")
                    with nc.allow_low_precision(
                        reason="denominator reciprocal, 18 bits is plenty"
                    ):
                        nc.vector.reciprocal_approx_fast(rec0, psden)
                    rec = sb.tile([1, QT], F32, tag="rec", bufs=2,
                                  name="rec")
                    nc.vector.tensor_copy(_r(rec), rec0)
                    return (psout_p, rec, h_p)

                def norm_b(normed):
                    psout_p, rec, h_p = normed
                    psr = psum.tile([P, QT], F32, tag="stat", bufs=3,
                                    name="psr")
                    nc.tensor.matmul(psr, lhsT=_r(ones_row), rhs=_r(rec))
                    nc.vector.tensor_copy(ot_res[:, h_p], psout_p)
                    nc.vector.tensor_tensor(ot_res[:, h_p], ot_res[:, h_p],
                                            psr, op=OP.mult)

                ready = []
                for h in range(NHL):
                    vsl = vsl_views[(t * NHL + h) % 2]
                    nc.sync.dma_start_transpose(
                        vsl[:, ds(0, n_slots)],
                        vT[ds(h * HD, HD), ds(0, n_slots * P)],
                    )
                    ksl = k_res[:, h].rearrange("p (c x) -> p c x", x=P)
                    # head h-2's psr broadcast FIRST: frees its psout so
                    # at most 2 psout + 3 score tiles are live in the
                    # 5-deep PSUM ring, and the h-2 reciprocal has had a
                    # full head of DVE slack
                    if len(ready) >= 2:
                        norm_b(ready.pop(0))
                    psout = psum.tile([P, QT], F32, tag="mm", bufs=5,
                                      name="psout")
                    psden = psum.tile([1, QT], F32, tag="stat", bufs=3,
                                      name="psden")
                    qsl = q_res[:, h, ds(qsl_off, QT)]

                    ets = {}

                    def emit_score(s):
                        pss = psum.tile([P, QT], F32, tag="mm", bufs=5,
                                        name="pss")
                        nc.tensor.matmul(pss, lhsT=ksl[:, s], rhs=qsl)
                        et = sb.tile([P, QT], BF16, tag="exp", bufs=4,
                                     name="et")
                        nc.scalar.activation(et, pss, AF.Exp,
                                             bias=nbias[:, ds(0, 1)])
                        if s >= 4 * t:
                            nc.vector.tensor_tensor(et, et,
                                                    masks[:, s - 4 * t],
                                                    op=OP.mult)
                        ets[s] = et

                    # scores first so Scalar starts on the exps, THEN the
                    # previous tile's out-projection (pure PE work) fills
                    # the exp latency before the PV stream needs et(0)
                    for s in range(min(LOOKAHEAD, n_slots)):
                        emit_score(s)
                    if prev_ot is not None:
                        outproj_chunk(prev_ot, prev_qoff, 2 * h)
                        outproj_chunk(prev_ot, prev_qoff, 2 * h + 1)
                    for s in range(n_slots):
                        if s + LOOKAHEAD < n_slots:
                            emit_score(s + LOOKAHEAD)
                        et = ets.pop(s)
                        nc.tensor.matmul(
                            psout,
                            lhsT=vsl[:, s],
                            rhs=et,
                            start=(s == 0),
                            stop=(s == n_slots - 1),
                        )
                        # softmax denominator: PE PSUM-accumulated ones
                        # reduction (the vector engines can't keep up
                        # with chained adds; the PE absorbs it for 512
                        # rows/slot with no cross-engine serialization)
                        nc.tensor.matmul(
                            psden,
                            lhsT=ones_bf,
                            rhs=et,
                            start=(s == 0),
                            stop=(s == n_slots - 1),
                        )
                    # this head's reciprocal: DVE has all of head h+1 to
                    # finish it before norm_b(h) runs at the top of h+2
                    ready.append(norm_a((psout, psden, h)))
                for r in ready:
                    norm_b(r)
                prev_ot = ot_res
                prev_qoff = qsl_off

            # ---- out-projection for the last q tile ----
            for e in range(NH):
                outproj_chunk(prev_ot, prev_qoff, e)

    nc.compile()
    return nc


# --------------------------------------------------------------------------
# Host-side prep and driver
# --------------------------------------------------------------------------


def make_host_data(x, w_in, w_out, q_gamma, q_beta, k_gamma, k_beta):
    """Build per-core in_maps (list of dicts). Core c: batch c//2, heads
    (c%2)*8..(c%2)*8+7."""
    import ml_dtypes
    bf16 = ml_dtypes.bfloat16

    B = x.shape[0]
    n_cores = 2 * B
    HF = NHL * P  # 1024 local head-features

    # LN gamma/beta are ones/zeros by the problem spec; the kernel folds
    # the q-side 1/sqrt(HD) score scale into the q projection rows and
    # drops the affine entirely.
    assert np.allclose(np.asarray(q_gamma, np.float32), 1.0)
    assert np.allclose(np.asarray(k_gamma, np.float32), 1.0)
    assert np.allclose(np.asarray(q_beta, np.float32), 0.0)
    assert np.allclose(np.asarray(k_beta, np.float32), 0.0)

    w32 = np.asarray(w_in, np.float32)
    wq = w32[0:D]
    wk = w32[D:2 * D]
    wv = w32[2 * D:3 * D]
    # centering the output features makes projected q/k exactly zero-mean
    wq_c = (wq - wq.mean(axis=0, keepdims=True)) / math.sqrt(HD)
    wk_c = wk - wk.mean(axis=0, keepdims=True)
    woT = np.asarray(w_out, np.float32).T  # [D(hfeat), D(out)]

    inv = 1.0 / (10000.0 ** (np.arange(0, HD, 2, dtype=np.float64) / HD))
    tpos = np.arange(S, dtype=np.float64)
    fr = np.outer(tpos, inv)
    emb = np.concatenate([fr, fr], axis=-1)  # [S, HD]
    cosT = np.ascontiguousarray(np.cos(emb).T).astype(bf16)  # [HD, S]
    sinT = np.ascontiguousarray(np.sin(emb).T).astype(bf16)

    h2 = HD // 2
    rotmT = np.zeros((P, P), np.float32)
    for p in range(h2):
        rotmT[p + h2, p] = -1.0
    for p in range(h2, HD):
        rotmT[p - h2, p] = 1.0
    rotm = rotmT.astype(bf16)

    # diagonal causal masks, identical for every 512-token tile:
    # mask[p, i, j] = (128*i + p) <= j
    pp = np.arange(P)[:, None, None]
    ii = np.arange(4)[None, :, None]
    jj = np.arange(QT)[None, None, :]
    masks = ((128 * ii + pp) <= jj).astype(np.float32).astype(bf16)

    scale = 1.0 / math.sqrt(HD)
    onesc = np.ones((P, 1), np.float32)
    onesr = np.ones((1, P), np.float32)

    role_data = []
    for r in range(2):
        fsl = slice(r * HF, (r + 1) * HF)
        wqkT = np.concatenate([wq_c[fsl].T, wk_c[fsl].T], axis=1)  # [D, 2HF]
        wqk_t = np.ascontiguousarray(
            wqkT.reshape(DC, P, 2 * NHL, P).transpose(2, 1, 0, 3)
        ).astype(bf16)
        wv_t = np.ascontiguousarray(
            wv[fsl].T.reshape(DC, P, NHL, P).transpose(2, 1, 0, 3)
        ).astype(bf16)
        wo_t = np.ascontiguousarray(
            woT[fsl].reshape(NHL, P, NH, P).transpose(2, 1, 0, 3)
        ).astype(bf16)
        role_data.append(dict(wqk=wqk_t, wv=wv_t, wo=wo_t))

    in_maps = []
    xT_b = {}
    for c in range(n_cores):
        b, r = c // 2, c % 2
        if b not in xT_b:
            xT_b[b] = np.ascontiguousarray(
                np.asarray(x[b], np.float32).T
            ).astype(bf16)
        in_maps.append(dict(
            xT=xT_b[b], cos=cosT, sin=sinT, masks=masks,
            onesc=onesc, onesr=onesr, rotm=rotm,
            **role_data[r],
        ))
    return in_maps


_PROGRAM_CACHE = {}


def _get_program():
    if "full" not in _PROGRAM_CACHE:
        _PROGRAM_CACHE["full"] = build_program()
    return _PROGRAM_CACHE["full"]


def run_full(x, w_in, w_out, q_gamma, q_beta, k_gamma, k_beta,
             trace=False):
    from concourse.bass_utils import run_bass_kernel_spmd

    B = x.shape[0]
    n_cores = 2 * B
    in_maps = make_host_data(
        x, w_in, w_out, q_gamma, q_beta, k_gamma, k_beta,
    )
    nc = _get_program()
    res = run_bass_kernel_spmd(
        nc, in_maps, core_ids=list(range(n_cores)), trace=trace,
    )
    out = np.empty((B, S, D), np.float32)
    for b in range(B):
        o = res.results[2 * b]["out"] + res.results[2 * b + 1]["out"]
        out[b] = o.T
    return out, res


def kernel(x, w_in, w_out, q_gamma, q_beta, k_gamma, k_beta, n_heads=16,
           **_ignored):
    x = np.asarray(x, np.float32)
    assert int(np.asarray(n_heads)) * HD == x.shape[-1]
    out, _ = run_full(
        np.asarray(x, np.float32),
        np.asarray(w_in, np.float32),
        np.asarray(w_out, np.float32),
        np.asarray(q_gamma, np.float32),
        np.asarray(q_beta, np.float32),
        np.asarray(k_gamma, np.float32),
        np.asarray(k_beta, np.float32),
    )
    return out
